# revision 9
# baseline (speedup 1.0000x reference)
"""Trainium2 Bass kernel for nn_Decoder (Gumbel-softmax hard sample + per-agent
2x2 linear), v4.

Contract: kernel(**inputs) takes FULL unsharded inputs, returns
(weights, actions) matching reference(). Internally shards data-parallel over
the agent axis N across 8 NeuronCores (125056 agents per core in a
[128 partitions x 977 cols] grid, agent row = p*977 + j).

Math (per agent n over M=64 abstract agents):
    idx = argmax_m [log(p/(1-p)) + gumbel(u)]
        = argmax_m y,   y = (exp(-ln p) - 1) * ln u   (monotone transform, <0)
    weights = sigmoid(W @ [idx, abs_actions[idx]] + b); actions = weights > 0

Design (all heavy compute on ScalarE + VectorE; GpSimd is avoided in the
streaming loop because DVE and GpSimd arbitrate for a shared SBUF port pair
and the loser blocks for the whole instruction):
  ScalarE: a = Ln(u); lp = Ln(p); rp = Exp(-lp)       (one LUT table set)
  VectorE: y = (rp - 1) * a  (fused scalar_tensor_tensor)
           embed a per-m 14-bit code into y's low mantissa bits:
           E = (y & ~0x3FFF) | (q8[m] << 6 | m), where q8 is a host-built
           8-bit uniform quantization of abs_actions (y is negative, so the
           fp32 max over each 64-block picks the argmax and carries the code;
           ties break to the smallest code = first occurrence).
           segmented reduce_max over each 64-wide block -> mx.
  Tail:    idx = mx & 63; A = float(mx & 0xFFC0) * (qscale/64) + qlo
           (no gather at all); 2x2 linear on VectorE; Sigmoid on ScalarE;
           actions = s > sigmoid-flush threshold.
  The 64 MB/core of p+u streams through a 3-deep tile pool; chunk DMAs are
  issued on the SP HWDGE ring and overlap compute.

Error budget: a few hundred of 1M rows sit within 2^-9 relative of the
runner-up key and may pick it (the reference's own fp path flips a handful
too); plus an 8-bit quantization of abs_actions. Measured L2 rel err ~5e-3
(gate 2e-2). Set SINGLE_REDUCE = False for a two-reduce variant (6-bit idx
embed + 12-bit A embed in separate reduces, rel err ~6e-4, ~15% slower).
"""

import numpy as np

import concourse.bass as bass
import concourse.mybir as mybir
import concourse.tile as tile
from concourse import bacc
from concourse.bass_utils import run_bass_kernel_spmd

# The act-table-load pass picks the first table set containing each function;
# blank all sets except the combined ln+exp set and the sigmoid set (indices
# preserved) so the kernel needs two table loads total.
_KEEP_SETS = {"natural_log_exp_and_others", "sigmoid_and_others"}
_orig_get_tables = bacc.get_activation_tables


def _patched_get_tables(arch):
    t = _orig_get_tables(arch)
    return {k: (v if k in _KEEP_SETS else set()) for k, v in t.items()}


bacc.get_activation_tables = _patched_get_tables

SINGLE_REDUCE = True

N_FULL = 1_000_000
M = 64
NCORES = 8
P = 128
J = 977                      # grid columns per partition per core
G = P * J                    # 125056 agents per core
N_PAD = NCORES * G           # 1000448
T2 = 88                      # grid columns per streaming chunk
BUFS = 3
F32 = mybir.dt.float32
I32 = mybir.dt.int32
U8 = mybir.dt.uint8

_CACHE = {}


def _build(hw_repeat=None, single_reduce=SINGLE_REDUCE,
           internal_inputs=False):
    L = mybir.ActivationFunctionType
    A = mybir.AluOpType
    X = mybir.AxisListType.X

    nc = bacc.Bacc("TRN2", target_bir_lowering=False, debug=False,
                   num_devices=NCORES)
    # internal_inputs: timing-only mode — big inputs live in device DRAM
    # scratch so wall-clock timing doesn't ship 0.5 GB per call
    ikind = "Internal" if internal_inputs else "ExternalInput"
    p_d = nc.dram_tensor("p_in", [G, M], F32, kind=ikind).ap()
    u_d = nc.dram_tensor("u_in", [G, M], F32, kind=ikind).ap()
    w_d = nc.dram_tensor("w_in", [G, 4], F32, kind=ikind).ap()
    b_d = nc.dram_tensor("b_in", [G, 2], F32, kind=ikind).ap()
    qa_d = nc.dram_tensor("qa_in", [P, M], I32, kind="ExternalInput").ap()
    qb_d = nc.dram_tensor("qb_in", [P, M], I32, kind="ExternalInput").ap()
    dec_d = nc.dram_tensor("dec_in", [P, 2], F32, kind="ExternalInput").ap()
    wout_d = nc.dram_tensor("w_out", [G, 2], F32, kind="ExternalOutput").ap()
    aout_d = nc.dram_tensor("a_out", [G, 2], U8, kind="ExternalOutput").ap()

    p_v = p_d.rearrange("(p j) m -> p j m", p=P)
    u_v = u_d.rearrange("(p j) m -> p j m", p=P)
    w_v = w_d.rearrange("(p j) k -> p (j k)", p=P)
    b_v = b_d.rearrange("(p j) k -> p (j k)", p=P)
    wout_v = wout_d.rearrange("(p j) k -> p (j k)", p=P)
    aout_v = aout_d.rearrange("(p j) k -> p (j k)", p=P)

    def chunks():
        out = []
        j = 0
        while j < J:
            t = min(T2, J - j)
            out.append((j, t))
            j += t
        return out

    with tile.TileContext(nc) as tc:
        with (
            tc.tile_pool(name="stream", bufs=12) as pool,
            tc.tile_pool(name="resident", bufs=1) as rpool,
        ):
            qat = rpool.tile([P, M], I32, tag="qat")
            nc.sync.dma_start(qat[:], qa_d[:])
            qbt = rpool.tile([P, M], I32, tag="qbt")
            nc.sync.dma_start(qbt[:], qb_d[:])
            dect = rpool.tile([P, 2], F32, tag="dect")
            nc.sync.dma_start(dect[:], dec_d[:])
            c_mask = rpool.tile([P, 1], I32, tag="c_mask")
            nc.vector.memset(c_mask[:], -16384 if single_reduce else -64)
            c_maskA = rpool.tile([P, 1], I32, tag="c_maskA")
            nc.vector.memset(c_maskA[:], -4096)
            c_63 = rpool.tile([P, 1], I32, tag="c_63")
            nc.vector.memset(c_63[:], 63)
            c_q = rpool.tile([P, 1], I32, tag="c_q")
            nc.vector.memset(c_q[:], 16320 if single_reduce else 4095)
            # W/b residents ride the gpsimd SWDGE queue so the 3 MB doesn't
            # sit ahead of chunk 0's loads on the SP HWDGE ring (they're
            # only needed in phase 2)
            Wsb = rpool.tile([P, J * 4], F32, tag="Wsb")
            nc.gpsimd.dma_start(Wsb[:], w_v)
            bsb = rpool.tile([P, J * 2], F32, tag="bsb")
            nc.gpsimd.dma_start(bsb[:], b_v)
            mx1 = rpool.tile([P, J], F32, tag="mx1")
            if single_reduce:
                mx2 = mx1
            else:
                mx2 = rpool.tile([P, J], F32, tag="mx2")

            def body():
                # ---- phase 1: streaming embedded argmax over 64-blocks ----
                for (j0, t) in chunks():
                    f = t * M
                    # asymmetric depth: pt one buffer deeper hides more DMA
                    # completion latency (p is consumed earlier than u)
                    pt = pool.tile([P, T2 * M], F32, tag="pt", bufs=4)
                    ut = pool.tile([P, T2 * M], F32, tag="ut", bufs=3)
                    nc.sync.dma_start(
                        pt[:, :f].rearrange("p (t m) -> p t m", m=M),
                        p_v[:, j0:j0 + t, :])
                    nc.sync.dma_start(
                        ut[:, :f].rearrange("p (t m) -> p t m", m=M),
                        u_v[:, j0:j0 + t, :])
                    # ACT: a = Ln(u); lp = Ln(p); rp = Exp(-lp)  (in place)
                    nc.scalar.activation(ut[:, :f], ut[:, :f], L.Ln)
                    nc.scalar.activation(pt[:, :f], pt[:, :f], L.Ln)
                    nc.scalar.activation(pt[:, :f], pt[:, :f], L.Exp,
                                         scale=-1.0)
                    # DVE: y = (rp - 1) * a
                    nc.vector.scalar_tensor_tensor(
                        ut[:, :f], pt[:, :f], -1.0, ut[:, :f],
                        op0=A.add, op1=A.mult)
                    yi3 = ut[:, :f].bitcast(I32).rearrange(
                        "p (t m) -> p t m", m=M)
                    u3 = ut[:, :f].rearrange("p (t m) -> p t m", m=M)
                    # DVE: E = (y & mask) | code;  segmented max
                    nc.vector.scalar_tensor_tensor(
                        yi3, yi3, c_mask[:],
                        qat[:].unsqueeze(1).broadcast_to([P, t, M]),
                        op0=A.bitwise_and, op1=A.bitwise_or)
                    nc.vector.tensor_reduce(
                        mx1[:, j0:j0 + t], u3, axis=X, op=A.max)
                    if not single_reduce:
                        # second embed: (E & ~4095) | q12  (bits 0..11 of E
                        # hold junk from embed1's idx; the and clears them)
                        nc.vector.scalar_tensor_tensor(
                            yi3, yi3, c_maskA[:],
                            qbt[:].unsqueeze(1).broadcast_to([P, t, M]),
                            op0=A.bitwise_and, op1=A.bitwise_or)
                        nc.vector.tensor_reduce(
                            mx2[:, j0:j0 + t], u3, axis=X, op=A.max)

                # ---- phase 2: decode + 2x2 linear + sigmoid ----
                idxt = rpool.tile([P, J], I32, tag="idxt")
                nc.vector.tensor_scalar(
                    idxt[:], mx1[:].bitcast(I32), c_63[:], None,
                    op0=A.bitwise_and)
                idxf = rpool.tile([P, J], F32, tag="idxf")
                nc.gpsimd.tensor_copy(idxf[:], idxt[:])
                # SBUF aliasing: idxt's tile is dead after the idxf copy,
                # so reuse it for qt, then (as an f32 view) for agt; the
                # tile graph's WAR tracking orders the reuse
                qt = idxt
                qsrc = mx1 if single_reduce else mx2
                nc.vector.tensor_scalar(
                    qt[:], qsrc[:].bitcast(I32), c_q[:], None,
                    op0=A.bitwise_and)
                qf = rpool.tile([P, J], F32, tag="qf")
                nc.gpsimd.tensor_copy(qf[:], qt[:])
                agt = idxt[:].bitcast(F32)
                nc.vector.tensor_scalar(
                    agt, qf[:], dect[:, 0:1], dect[:, 1:2],
                    op0=A.mult, op1=A.add)

                st = rpool.tile([P, J * 2], F32, tag="st")
                sv = st[:].rearrange("p (j k) -> p j k", k=2)
                Wv = Wsb[:].rearrange("p (j k) -> p j k", k=4)
                bv = bsb[:].rearrange("p (j k) -> p j k", k=2)
                tmp0 = mx1  # mx1 is dead after the decode extracts
                tmp1 = qf   # qf is dead after agt; reuse to save SBUF
                for o in range(2):
                    nc.vector.tensor_tensor(
                        tmp0[:], Wv[:, :, 2 * o], idxf[:], op=A.mult)
                    nc.vector.tensor_tensor(
                        tmp1[:], Wv[:, :, 2 * o + 1], agt, op=A.mult)
                    nc.vector.tensor_tensor(tmp0[:], tmp0[:], tmp1[:],
                                            op=A.add)
                    nc.vector.tensor_tensor(
                        sv[:, :, o], tmp0[:], bv[:, :, o], op=A.add)

                # threshold on s first, then sigmoid in place: saves a
                # [P, 2J] tile. reference actions = (sigmoid(s) > 0);
                # jax-on-neuron's logistic flushes to 0 iff s < -128*ln2.
                au = rpool.tile([P, J * 2], U8, tag="au")
                nc.vector.tensor_scalar(au[:], st[:], -88.7228390619, None,
                                        op0=A.is_gt)
                nc.scalar.activation(st[:], st[:], L.Sigmoid)

                nc.sync.dma_start(wout_v, st[:])
                nc.sync.dma_start(aout_v, au[:])

            if hw_repeat is not None:
                if hw_repeat > 0:
                    with tc.For_i(0, hw_repeat):
                        body()
                # hw_repeat == 0: empty module (overhead calibration)
            else:
                body()

    nc.compile()
    return nc


def _codebook(abs_actions, single_reduce=SINGLE_REDUCE):
    """Host-side quantized codebooks for abs_actions.

    single-reduce: qa = (q10 << 6) | m, dequant scale pre-divided by 64
    (decode keeps q << 6). two-reduce: qa = iota (6-bit idx embed),
    qb = q12 (12-bit value embed)."""
    Aa = np.asarray(abs_actions, dtype=np.float32)
    lo = float(Aa.min())
    hi = float(Aa.max())
    steps = 255.0 if single_reduce else 4095.0
    scale = (hi - lo) / steps if hi > lo else 1.0
    q = np.clip(np.round((Aa - lo) / scale).astype(np.int32), 0, int(steps))
    m = np.arange(M, dtype=np.int32)
    if single_reduce:
        qa = (q << 6) | m
        qb = np.zeros(M, dtype=np.int32)
        dec = np.array([[scale / 64.0, lo]], dtype=np.float32)
    else:
        qa = m
        qb = q
        dec = np.array([[scale, lo]], dtype=np.float32)
    return (np.tile(qa[None, :], (P, 1)).astype(np.int32),
            np.tile(qb[None, :], (P, 1)).astype(np.int32),
            np.tile(dec, (P, 1)))


def prepare(partition, abs_actions, u, W, b):
    """Build (or reuse) the Bass module and the per-core input maps."""
    partition = np.asarray(partition, dtype=np.float32)
    u = np.asarray(u, dtype=np.float32)
    W = np.asarray(W, dtype=np.float32)
    b = np.asarray(b, dtype=np.float32)

    if "nc" not in _CACHE:
        _CACHE["nc"] = _build()
    nc = _CACHE["nc"]

    qa, qb, dec = _codebook(abs_actions)
    w_flat = np.ascontiguousarray(W.reshape(N_FULL, 4))
    b_flat = np.ascontiguousarray(b)

    def shard(arr, fill):
        shards = []
        for k in range(NCORES):
            lo_, hi_ = k * G, (k + 1) * G
            if hi_ <= N_FULL:
                shards.append(arr[lo_:hi_])
            else:
                padrows = np.full((hi_ - N_FULL, arr.shape[1]), fill,
                                  dtype=np.float32)
                shards.append(np.concatenate([arr[lo_:N_FULL], padrows],
                                             axis=0))
        return shards

    p_s = shard(partition, 0.5)
    u_s = shard(u, 0.5)
    w_s = shard(w_flat, 0.0)
    b_s = shard(b_flat, 0.0)

    in_maps = [
        {"p_in": p_s[k], "u_in": u_s[k], "w_in": w_s[k], "b_in": b_s[k],
         "qa_in": qa, "qb_in": qb, "dec_in": dec}
        for k in range(NCORES)
    ]
    return nc, in_maps


def kernel(partition, abs_actions, u, W, b):
    nc, in_maps = prepare(partition, abs_actions, u, W, b)

    res = run_bass_kernel_spmd(nc, in_maps, core_ids=list(range(NCORES)))

    weights = np.concatenate([res.results[k]["w_out"] for k in range(NCORES)],
                             axis=0)[:N_FULL]
    actions = np.concatenate([res.results[k]["a_out"] for k in range(NCORES)],
                             axis=0)[:N_FULL].astype(bool)
    return weights, actions
